# revision 5
# baseline (speedup 1.0000x reference)
"""Bass/Trainium2 kernel for nn_EnergyOutputCollector.

Math (per batch row b):
    w[c]      = position_weights.flat[cell_ids[c]]
    surface   = scatter(energy * w) -> [B, 1024]   (cell_ids is a permutation)
    h1 = LN(gelu_tanh(surface @ W1 + b1)) * g1 + bb1
    h2 = LN(gelu_tanh(h1 @ W2 + b2)) * g2 + bb2
    out = h2 @ W3 + b3

Strategy:
  - Data-parallel: batch (16384) split across 8 NeuronCores (2048 each).
  - The scatter + position-weight gather + LN affine params fold into the
    weights on the host in exact fp32:
        W1' = (w[:,None] * W1[cell_ids])          (scatter == row gather of W1)
        W2' = diag(g1) @ W2,  bias2' = bb1 @ W2 + b2
        W3' = diag(g2) @ W3,  bias3' = bb2 @ W3 + b3
  - Matmuls run on the PE in fp8 e4m3 DoubleRow mode (2 k-chunks per
    instruction, 0.5 cycles/row = 4x the fp16 rate).  To recover ~fp16
    accuracy each operand is split hi/lo:  x ~= xhi + xlo with
    xhi = fp8(x), xlo = fp8(x - xhi), and the product takes 3 of the 4
    terms  (xhi*whi + xlo*whi + xhi*wlo),  i.e. 0.75x the fp16 PE time.
  - Power-of-2 scaling keeps fp8 operands out of the subnormal range:
    activations x16 (folded into the LN rstd), weights x64 (host), and
    the 2^-10 correction folds into the gelu / output-copy scale.
  - fp32 PSUM accumulation; gelu on ScalarE straight from PSUM with the
    2^-10 scale; LayerNorm stats via bn_stats/bn_aggr on VectorE; one
    batched fp16 SBUF->SBUF DMA transpose per layer output; then the
    hi cast on ScalarE (Copy, same ACT table set as gelu -> no swap)
    and the lo residual on VectorE (tensor_tensor subtract, fp8 out).
  - Batch tiles processed layer-major in groups of 4 so ScalarE sqrt ops
    (LN rstd) batch together -- ACT table-set swaps cost ~1.3-2.7us each.
"""

import numpy as np
import ml_dtypes

import concourse.bass as bass
import concourse.mybir as mybir
import concourse.tile as tile
from concourse import bacc
from concourse.bass_utils import run_bass_kernel_spmd

N_CORES = 8
SURF = 1024
HID = 2048
INTER = 2048
OUT = 768
BATCH = 16384
BC = BATCH // N_CORES          # batch per core
MT = BC // 128                 # m-tiles per core (16)
GROUP = 4                      # m-tiles per layer-major group
EPS = 1e-5

ASCALE = 16.0                  # activation fp8 scale (2^4)
WSCALE = 64.0                  # weight fp8 scale (2^6)
PSCALE = 1.0 / (ASCALE * WSCALE)   # psum -> true scale (2^-10)

F = mybir.ActivationFunctionType
ALU = mybir.AluOpType
DR = mybir.MatmulPerfMode.DoubleRow
F8 = mybir.dt.float8e4
F16 = mybir.dt.float16
F32 = mybir.dt.float32
NP_F8 = ml_dtypes.float8_e4m3

_PROGRAM_CACHE: dict = {}
_LAST_EXEC_NS = None


def _build_program(with_b1: bool, with_b2: bool, with_b3: bool, repeats: int = 1,
                   group: int = GROUP):
    assert MT % group == 0, (MT, group)
    nc = bacc.Bacc(None, target_bir_lowering=False, debug=False)

    ehi = nc.dram_tensor("ehi", [SURF, BC], F8, kind="ExternalInput")
    elo = nc.dram_tensor("elo", [SURF, BC], F8, kind="ExternalInput")
    w1hi = nc.dram_tensor("w1hi", [SURF, HID], F8, kind="ExternalInput")
    w1lo = nc.dram_tensor("w1lo", [SURF, HID], F8, kind="ExternalInput")
    w2hi = nc.dram_tensor("w2hi", [HID, INTER], F8, kind="ExternalInput")
    w2lo = nc.dram_tensor("w2lo", [HID, INTER], F8, kind="ExternalInput")
    w3hi = nc.dram_tensor("w3hi", [INTER, OUT], F8, kind="ExternalInput")
    w3lo = nc.dram_tensor("w3lo", [INTER, OUT], F8, kind="ExternalInput")
    b1d = nc.dram_tensor("b1", [HID], F32, kind="ExternalInput") if with_b1 else None
    b2d = nc.dram_tensor("b2", [INTER], F32, kind="ExternalInput") if with_b2 else None
    b3d = nc.dram_tensor("b3", [OUT], F32, kind="ExternalInput") if with_b3 else None
    out = nc.dram_tensor("out", [BC, OUT], F32, kind="ExternalOutput")

    # K-on-partitions layouts for matmul operands
    ehit = ehi.rearrange("(ko p) b -> p ko b", p=128)     # [128, 8, BC]
    elot = elo.rearrange("(ko p) b -> p ko b", p=128)
    w1s = [w.rearrange("(ko p) n -> p ko n", p=128) for w in (w1hi, w1lo)]
    w2s = [w.rearrange("(ko p) n -> p ko n", p=128) for w in (w2hi, w2lo)]
    w3s = [w.rearrange("(ko p) n -> p ko n", p=128) for w in (w3hi, w3lo)]

    with tile.TileContext(nc) as tc:
        with (
            tc.tile_pool(name="weights", bufs=1) as wp,
            tc.tile_pool(name="consts", bufs=1) as cp,
            tc.tile_pool(name="etiles", bufs=group + 2) as ep,
            tc.tile_pool(name="etilesl", bufs=group + 2) as epl,
            tc.tile_pool(name="acts", bufs=group + 1) as hp,
            tc.tile_pool(name="actsT", bufs=group + 1) as tp,
            tc.tile_pool(name="actsHi", bufs=group + 1) as qhp,
            tc.tile_pool(name="actsLo", bufs=group + 1) as qlp,
            tc.tile_pool(name="stats", bufs=2 * group) as sp,
            tc.tile_pool(name="outs", bufs=2) as op,
            tc.tile_pool(name="psum", bufs=2, space="PSUM") as pp,
        ):
            w1_sb = [wp.tile([128, 8, HID], F8, tag=f"w1{i}", name=f"w1_{i}")
                     for i in range(2)]
            w2_sb = [wp.tile([128, 16, INTER], F8, tag=f"w2{i}", name=f"w2_{i}")
                     for i in range(2)]
            w3_sb = [wp.tile([128, 16, OUT], F8, tag=f"w3{i}", name=f"w3_{i}")
                     for i in range(2)]
            for sb, dr in zip(w1_sb + w2_sb + w3_sb, w1s + w2s + w3s):
                nc.sync.dma_start(sb[:], dr)

            eps_t = cp.tile([128, 1], F32, tag="eps")
            nc.vector.memset(eps_t[:], EPS / (ASCALE * ASCALE))

            def bias_bc(dram_vec, n, tag):
                t = cp.tile([128, n], F32, tag=tag)
                src = bass.AP(
                    tensor=dram_vec.tensor,
                    offset=dram_vec.offset,
                    ap=[[0, 128]] + list(dram_vec.ap),
                )
                nc.sync.dma_start(t[:], src)
                return t

            b1_sb = bias_bc(b1d[:], HID, "b1") if with_b1 else None
            b2_sb = bias_bc(b2d[:], INTER, "b2") if with_b2 else None
            b3_sb = bias_bc(b3d[:], OUT, "b3") if with_b3 else None

            def mlp_layer(lhsT_hi, lhsT_lo, w_sb, nk, bias_sb, tiles):
                """3-term fp8 DoubleRow matmul over nk k-chunks + optional
                bias + scaled gelu; returns gelu'd fp16 [128, 2048] tiles."""
                whi, wlo = w_sb
                hs = {}
                for t in tiles:
                    h = hp.tile([128, 2048], F16, tag="h")
                    ps = pp.tile([128, 2048], F32, tag="ps")
                    # stationary (lhsT) held across the 4 psum n-slices so
                    # the 256-row DoubleRow weight load amortizes over 4
                    # moving streams
                    terms = ((lhsT_hi, whi), (lhsT_lo, whi), (lhsT_hi, wlo))
                    for ti, (lhs_of, w) in enumerate(terms):
                        for kp in range(nk // 2):
                            first = ti == 0 and kp == 0
                            last = ti == len(terms) - 1 and kp == nk // 2 - 1
                            for n in range(4):
                                psl = slice(n * 512, (n + 1) * 512)
                                nc.tensor.matmul(
                                    ps[:, psl], lhs_of(t, kp),
                                    w[:, 2 * kp:2 * kp + 2, psl],
                                    start=first, stop=last,
                                    perf_mode=DR,
                                )
                    if bias_sb is not None:
                        nc.vector.tensor_add(out=ps[:], in0=ps[:], in1=bias_sb[:])
                    nc.scalar.activation(
                        out=h[:], in_=ps[:], func=F.Gelu_apprx_tanh, scale=PSCALE
                    )
                    hs[t] = h
                return hs

            def layernorm_transpose(hs, tiles):
                """LN (in place, scaled x16) then batched transpose and fp8
                hi/lo casts; returns (hi8, lo8) [128, 16, 128] tiles."""
                mvs = {}
                for t in tiles:
                    stats = sp.tile([128, 4, 6], F32, tag="stats")
                    for n in range(4):
                        nc.vector.bn_stats(
                            out=stats[:, n, :], in_=hs[t][:, n * 512:(n + 1) * 512]
                        )
                    mv = sp.tile([128, 2], F32, tag="mv")
                    nc.vector.bn_aggr(out=mv[:], in_=stats[:])
                    mvs[t] = mv
                rs = {}
                for t in tiles:   # batched: one ACT table-set swap per group
                    r = sp.tile([128, 1], F32, tag="rstd")
                    # sqrt(var/A^2 + eps/A^2) = std/A ; reciprocal -> A*rstd
                    nc.scalar.activation(
                        out=r[:], in_=mvs[t][:, 1:2], func=F.Sqrt, bias=eps_t[:],
                        scale=1.0 / (ASCALE * ASCALE),
                    )
                    rs[t] = r
                hts = {}
                for t in tiles:
                    nc.vector.reciprocal(out=rs[t][:], in_=rs[t][:])
                    nc.vector.tensor_scalar(
                        out=hs[t][:], in0=hs[t][:],
                        scalar1=mvs[t][:, 0:1], scalar2=rs[t][:],
                        op0=ALU.subtract, op1=ALU.mult,
                    )
                    ht = tp.tile([128, 16, 128], F16, tag="ht")
                    nc.sync.dma_start_transpose(ht[:], hs[t][:])
                    hts[t] = ht
                res = {}
                for t in tiles:
                    hi8 = qhp.tile([128, 16, 128], F8, tag="hi8")
                    lo8 = qlp.tile([128, 16, 128], F8, tag="lo8")
                    nc.scalar.activation(out=hi8[:], in_=hts[t][:], func=F.Copy)
                    nc.vector.tensor_tensor(
                        out=lo8[:], in0=hts[t][:], in1=hi8[:], op=ALU.subtract
                    )
                    res[t] = (hi8, lo8)
                return res

            def _full_body():
                for g in range(MT // group):
                    tiles = list(range(g * group, (g + 1) * group))

                    e_sbs = {}
                    for t in tiles:
                        e_hi = ep.tile([128, 8, 128], F8, tag="ehi")
                        e_lo = epl.tile([128, 8, 128], F8, tag="elo")
                        bsl = slice(t * 128, (t + 1) * 128)
                        nc.sync.dma_start(e_hi[:], ehit[:, :, bsl])
                        nc.sync.dma_start(e_lo[:], elot[:, :, bsl])
                        e_sbs[t] = (e_hi, e_lo)

                    h1 = mlp_layer(
                        lambda t, kp: e_sbs[t][0][:, 2 * kp:2 * kp + 2, :],
                        lambda t, kp: e_sbs[t][1][:, 2 * kp:2 * kp + 2, :],
                        w1_sb, 8, b1_sb, tiles,
                    )
                    h1q = layernorm_transpose(h1, tiles)
                    h2 = mlp_layer(
                        lambda t, kp: h1q[t][0][:, 2 * kp:2 * kp + 2, :],
                        lambda t, kp: h1q[t][1][:, 2 * kp:2 * kp + 2, :],
                        w2_sb, 16, b2_sb, tiles,
                    )
                    h2q = layernorm_transpose(h2, tiles)

                    w3hi_sb, w3lo_sb = w3_sb
                    for t in tiles:
                        ps = pp.tile([128, 2048], F32, tag="ps", name="ps_l3")
                        xhi, xlo = h2q[t]
                        terms3 = ((xhi, w3hi_sb), (xlo, w3hi_sb), (xhi, w3lo_sb))
                        for ti, (x, w) in enumerate(terms3):
                            for kp in range(8):
                                first = ti == 0 and kp == 0
                                last = ti == 2 and kp == 7
                                for n in range(2):
                                    bsl = slice(n * 512, n * 512 + 384)
                                    nsl = slice(n * 384, (n + 1) * 384)
                                    nc.tensor.matmul(
                                        ps[:, bsl], x[:, 2 * kp:2 * kp + 2, :],
                                        w[:, 2 * kp:2 * kp + 2, nsl],
                                        start=first, stop=last,
                                        perf_mode=DR,
                                    )
                        ps3 = ps[:, :1024].rearrange("p (b f) -> p b f", f=512)[:, :2, :384]
                        o_sb = op.tile([128, 2, 384], F32, tag="o")
                        if b3_sb is not None:
                            nc.vector.tensor_scalar(
                                out=o_sb[:], in0=ps3, scalar1=PSCALE, scalar2=0.0,
                                op0=ALU.mult, op1=ALU.add,
                            )
                            nc.vector.tensor_tensor(
                                out=o_sb[:], in0=o_sb[:],
                                in1=b3_sb[:].rearrange("p (b f) -> p b f", f=384),
                                op=ALU.add,
                            )
                        else:
                            nc.scalar.mul(o_sb[:], ps3, PSCALE)
                        nc.sync.dma_start(
                            out[t * 128:(t + 1) * 128, :],
                            o_sb[:].rearrange("p b f -> p (b f)"),
                        )

            if repeats == 1:
                _full_body()
            else:
                with tc.For_i(0, repeats, 1):
                    _full_body()

    nc.compile()
    return nc


def _split8(x):
    """fp8 e4m3 hi/lo split of an fp32 array (already scaled)."""
    hi = np.asarray(x, NP_F8)
    lo = np.asarray(x - hi.astype(np.float32), NP_F8)
    return hi, lo


def _prepare(energy, cell_ids, position_weights, W1, b1, ln1_g, ln1_b,
             W2, b2, ln2_g, ln2_b, W3, b3):
    """Host-side prep: shard + fold scatter/gather/LN-affine into weights,
    then fp8 hi/lo quantization with power-of-2 scaling.
    Returns (bias_flags_key, per-core input maps)."""
    energy = np.asarray(energy, dtype=np.float32)
    cell_ids = np.asarray(cell_ids)
    position_weights = np.asarray(position_weights, dtype=np.float32)
    W1 = np.asarray(W1, dtype=np.float32)
    W2 = np.asarray(W2, dtype=np.float32)
    W3 = np.asarray(W3, dtype=np.float32)
    b1 = np.asarray(b1, dtype=np.float32)
    b2 = np.asarray(b2, dtype=np.float32)
    b3 = np.asarray(b3, dtype=np.float32)
    ln1_g = np.asarray(ln1_g, dtype=np.float32)
    ln1_b = np.asarray(ln1_b, dtype=np.float32)
    ln2_g = np.asarray(ln2_g, dtype=np.float32)
    ln2_b = np.asarray(ln2_b, dtype=np.float32)

    ids = cell_ids.astype(np.int64)
    # scatter surface[:, ids] = (energy * w).T  ==  row-gather of W1 at ids
    # (ids is a permutation: fill=arange per the problem spec)
    w = position_weights.reshape(-1)[ids]
    W1f = w[:, None] * W1[ids]

    # fold LN affine params into the next layer (exact fp32 host math):
    # (xn*g + lb) @ W + b  ==  xn @ (diag(g) W) + (lb @ W + b)
    W2f = ln1_g[:, None] * W2
    b2f = ln1_b @ W2 + b2
    W3f = ln2_g[:, None] * W3
    b3f = ln2_b @ W3 + b3

    with_b1 = bool(np.any(b1 != 0.0))
    with_b2 = bool(np.any(b2f != 0.0))
    with_b3 = bool(np.any(b3f != 0.0))
    key = (with_b1, with_b2, with_b3)

    w1p = _split8(W1f * WSCALE)
    w2p = _split8(W2f * WSCALE)
    w3p = _split8(W3f * WSCALE)
    base = {
        "w1hi": w1p[0], "w1lo": w1p[1],
        "w2hi": w2p[0], "w2lo": w2p[1],
        "w3hi": w3p[0], "w3lo": w3p[1],
    }
    # biases enter the psum (which carries a 2^10 scale) pre-activation
    if with_b1:
        base["b1"] = b1 * (ASCALE * WSCALE)
    if with_b2:
        base["b2"] = b2f * (ASCALE * WSCALE)
    if with_b3:
        base["b3"] = b3f
    ehi, elo = _split8(energy * ASCALE)
    in_maps = [
        {**base,
         "ehi": np.ascontiguousarray(ehi[:, c * BC:(c + 1) * BC]),
         "elo": np.ascontiguousarray(elo[:, c * BC:(c + 1) * BC])}
        for c in range(N_CORES)
    ]
    return key, in_maps


def kernel(energy, cell_ids, position_weights, W1, b1, ln1_g, ln1_b,
           W2, b2, ln2_g, ln2_b, W3, b3):
    key, in_maps = _prepare(energy, cell_ids, position_weights, W1, b1,
                            ln1_g, ln1_b, W2, b2, ln2_g, ln2_b, W3, b3)
    if key not in _PROGRAM_CACHE:
        _PROGRAM_CACHE[key] = _build_program(*key)
    nc = _PROGRAM_CACHE[key]
    res = run_bass_kernel_spmd(nc, in_maps, core_ids=list(range(N_CORES)))
    global _LAST_EXEC_NS
    if res.exec_time_ns is not None:
        _LAST_EXEC_NS = res.exec_time_ns
    return np.concatenate([r["out"] for r in res.results], axis=0)


# revision 8
# speedup vs baseline: 1.4723x; 1.4723x over previous
"""Bass/Trainium2 kernel for nn_EnergyOutputCollector.

Math (per batch row b):
    w[c]      = position_weights.flat[cell_ids[c]]
    surface   = scatter(energy * w) -> [B, 1024]   (cell_ids is a permutation)
    h1 = LN(gelu_tanh(surface @ W1 + b1)) * g1 + bb1
    h2 = LN(gelu_tanh(h1 @ W2 + b2)) * g2 + bb2
    out = h2 @ W3 + b3

Strategy:
  - Data-parallel: batch (16384) split across 8 NeuronCores (2048 each).
  - Host folds (exact fp32):
      * scatter + position-weight gather -> row gather/scale of W1
      * LN affine (g, b) -> following weight/bias
      * LN mean subtraction -> COLUMN-CENTERING of the following weights:
            (h - mu 1) @ W == h @ (W - ones colmean(W) HID)
        so the device never materializes (h - mu).
  - The remaining per-row 1/std rides the NEXT layer's ScalarE op as a
    per-partition scale AP:  gelu(psum * rstd)  /  copy(psum * rstd).
    The DVE LayerNorm apply op disappears entirely, and the transpose
    runs on the RAW gelu output, shortening the PE's inter-layer
    dependency chain to gelu -> transpose (~3.5us), which the next
    3 tiles of matmul work fully covers.
  - Device: pure 3-layer MLP in fp16 (PE full rate), fp32 PSUM
    accumulation, gelu on ScalarE straight from PSUM, LayerNorm stats
    via bn_stats/bn_aggr on VectorE, rstd via a single DVE
    tensor_scalar pow(-1/2) (no ACT table swap), one batched fp16
    SBUF->SBUF DMA transpose per layer output.
"""

import numpy as np

import concourse.bass as bass
import concourse.mybir as mybir
import concourse.tile as tile
from concourse import bacc
from concourse.bass_utils import run_bass_kernel_spmd

N_CORES = 8
SURF = 1024
HID = 2048
INTER = 2048
OUT = 768
BATCH = 16384
BC = BATCH // N_CORES          # batch per core
MT = BC // 128                 # m-tiles per core (16)
GROUP = 4                      # m-tiles per layer-major group
EPS = 1e-5
RSTD_POW = False               # DVE pow(-0.5) fails walrus ISA check -> ACT sqrt

F = mybir.ActivationFunctionType
ALU = mybir.AluOpType
F16 = mybir.dt.float16
F32 = mybir.dt.float32

_PROGRAM_CACHE: dict = {}
_LAST_EXEC_NS = None


def _build_program(with_b1: bool, with_b2: bool, with_b3: bool, repeats: int = 1,
                   group: int = GROUP):
    assert MT % group == 0, (MT, group)
    nc = bacc.Bacc(None, target_bir_lowering=False, debug=False)

    e = nc.dram_tensor("e", [SURF, BC], F16, kind="ExternalInput")
    w1 = nc.dram_tensor("w1", [SURF, HID], F16, kind="ExternalInput")
    w2 = nc.dram_tensor("w2", [HID, INTER], F16, kind="ExternalInput")
    w3 = nc.dram_tensor("w3", [INTER, OUT], F16, kind="ExternalInput")
    b1d = nc.dram_tensor("b1", [HID], F16, kind="ExternalInput") if with_b1 else None
    b2d = nc.dram_tensor("b2", [INTER], F16, kind="ExternalInput") if with_b2 else None
    b3d = nc.dram_tensor("b3", [OUT], F32, kind="ExternalInput") if with_b3 else None
    out = nc.dram_tensor("out", [BC, OUT], F32, kind="ExternalOutput")

    # K-on-partitions layouts for matmul operands
    et = e.rearrange("(ko p) b -> p ko b", p=128)      # [128, 8, BC]
    w1t = w1.rearrange("(ko p) n -> p ko n", p=128)    # [128, 8, HID]
    w2t = w2.rearrange("(ko p) n -> p ko n", p=128)    # [128, 16, INTER]
    w3t = w3.rearrange("(ko p) n -> p ko n", p=128)    # [128, 16, OUT]

    with tile.TileContext(nc) as tc:
        with (
            tc.tile_pool(name="weights", bufs=1) as wp,
            tc.tile_pool(name="consts", bufs=1) as cp,
            tc.tile_pool(name="etiles", bufs=group + 2) as ep,
            tc.tile_pool(name="acts", bufs=group + 2) as hp,
            tc.tile_pool(name="actsT", bufs=group + 2) as tp,
            tc.tile_pool(name="stats", bufs=2 * group) as sp,
            tc.tile_pool(name="outs", bufs=min(group, 4)) as op,
            tc.tile_pool(name="psum", bufs=2, space="PSUM") as pp,
        ):
            w1_sb = wp.tile([128, 8, HID], F16, tag="w1")
            w2_sb = wp.tile([128, 16, INTER], F16, tag="w2")
            w3_sb = wp.tile([128, 16, OUT], F16, tag="w3")
            nc.sync.dma_start(w1_sb[:], w1t)
            nc.sync.dma_start(w2_sb[:], w2t)
            nc.sync.dma_start(w3_sb[:], w3t)

            eps_t = cp.tile([128, 1], F32, tag="eps")
            nc.vector.memset(eps_t[:], EPS)

            def bias_bc(dram_vec, n, tag, dt=F32):
                t = cp.tile([128, n], dt, tag=tag)
                src = bass.AP(
                    tensor=dram_vec.tensor,
                    offset=dram_vec.offset,
                    ap=[[0, 128]] + list(dram_vec.ap),
                )
                nc.sync.dma_start(t[:], src)
                return t

            b1_sb = bias_bc(b1d[:], HID, "b1", F16) if with_b1 else None
            b2_sb = bias_bc(b2d[:], INTER, "b2", F16) if with_b2 else None
            b3_sb = bias_bc(b3d[:], OUT, "b3") if with_b3 else None

            def mlp_layer(lhsT_of, w_sb, nk, bias_sb, tiles, scales):
                """matmul over nk k-chunks + optional bias + gelu (with the
                previous layer's rstd as a per-partition ACT scale);
                returns gelu'd fp16 [128, 2048] tiles."""
                hs = {}
                for t in tiles:
                    h = hp.tile([128, 2048], F16, tag="h")
                    ps = pp.tile([128, 2048], F32, tag="ps")
                    for n in range(4):
                        psl = slice(n * 512, (n + 1) * 512)
                        for k in range(nk):
                            nc.tensor.matmul(
                                ps[:, psl], lhsT_of(t, k), w_sb[:, k, psl],
                                start=(k == 0), stop=(k == nk - 1),
                            )
                    if bias_sb is not None and scales is not None:
                        # gelu(ps*rstd + b): apply rstd on DVE, bias, plain gelu
                        nc.vector.tensor_scalar(
                            out=ps[:], in0=ps[:], scalar1=scales[t][:],
                            scalar2=None, op0=ALU.mult,
                        )
                        nc.vector.tensor_add(out=ps[:], in0=ps[:], in1=bias_sb[:])
                        nc.scalar.activation(
                            out=h[:], in_=ps[:], func=F.Gelu_apprx_tanh
                        )
                    else:
                        if bias_sb is not None:
                            nc.vector.tensor_add(
                                out=ps[:], in0=ps[:], in1=bias_sb[:]
                            )
                        nc.scalar.activation(
                            out=h[:], in_=ps[:], func=F.Gelu_apprx_tanh,
                            scale=(scales[t][:] if scales is not None else 1.0),
                        )
                    hs[t] = h
                return hs

            def ln_stats_transpose(hs, tiles):
                """Transpose raw gelu output (feeds next matmul) and compute
                LN rstd per tile (consumed by the next ACT op as scale).
                Returns (hT tiles, rstd tiles)."""
                hts = {}
                for t in tiles:
                    ht = tp.tile([128, 16, 128], F16, tag="ht")
                    nc.sync.dma_start_transpose(ht[:], hs[t][:])
                    hts[t] = ht
                mvs = {}
                for t in tiles:
                    stats = sp.tile([128, 4, 6], F32, tag="stats")
                    for n in range(4):
                        nc.vector.bn_stats(
                            out=stats[:, n, :], in_=hs[t][:, n * 512:(n + 1) * 512]
                        )
                    mv = sp.tile([128, 2], F32, tag="mv")
                    nc.vector.bn_aggr(out=mv[:], in_=stats[:])
                    mvs[t] = mv
                rs = {}
                if RSTD_POW:
                    for t in tiles:
                        r = sp.tile([128, 1], F32, tag="rstd")
                        nc.vector.tensor_scalar(
                            out=r[:], in0=mvs[t][:, 1:2],
                            scalar1=EPS, scalar2=-0.5,
                            op0=ALU.add, op1=ALU.pow,
                        )
                        rs[t] = r
                else:
                    for t in tiles:   # batched: one ACT table swap per group
                        r = sp.tile([128, 1], F32, tag="rstd")
                        nc.scalar.activation(
                            out=r[:], in_=mvs[t][:, 1:2], func=F.Sqrt,
                            bias=eps_t[:],
                        )
                        rs[t] = r
                    for t in tiles:
                        nc.vector.reciprocal(out=rs[t][:], in_=rs[t][:])
                return hts, rs

            def _full_body():
                for g in range(MT // group):
                    tiles = list(range(g * group, (g + 1) * group))

                    e_sbs = {}
                    for t in tiles:
                        e_sb = ep.tile([128, 8, 128], F16, tag="e")
                        nc.sync.dma_start(
                            e_sb[:], et[:, :, t * 128:(t + 1) * 128]
                        )
                        e_sbs[t] = e_sb

                    h1 = mlp_layer(
                        lambda t, k: e_sbs[t][:, k, :], w1_sb, 8, b1_sb,
                        tiles, None,
                    )
                    h1T, rs1 = ln_stats_transpose(h1, tiles)
                    h2 = mlp_layer(
                        lambda t, k: h1T[t][:, k, :], w2_sb, 16, b2_sb,
                        tiles, rs1,
                    )
                    h2T, rs2 = ln_stats_transpose(h2, tiles)

                    for t in tiles:
                        ps = pp.tile([128, 2048], F32, tag="ps", name="ps_l3")
                        for n in range(2):
                            bsl = slice(n * 512, n * 512 + 384)
                            nsl = slice(n * 384, (n + 1) * 384)
                            for k in range(16):
                                nc.tensor.matmul(
                                    ps[:, bsl], h2T[t][:, k, :], w3_sb[:, k, nsl],
                                    start=(k == 0), stop=(k == 15),
                                )
                        ps3 = ps[:, :1024].rearrange("p (b f) -> p b f", f=512)[:, :2, :384]
                        o_sb = op.tile([128, 2, 384], F32, tag="o")
                        nc.scalar.activation(
                            out=o_sb[:], in_=ps3, func=F.Copy, scale=rs2[t][:]
                        )
                        if b3_sb is not None:
                            nc.vector.tensor_tensor(
                                out=o_sb[:], in0=o_sb[:],
                                in1=b3_sb[:].rearrange("p (b f) -> p b f", f=384),
                                op=ALU.add,
                            )
                        nc.sync.dma_start(
                            out[t * 128:(t + 1) * 128, :],
                            o_sb[:].rearrange("p b f -> p (b f)"),
                        )

            if repeats == 1:
                _full_body()
            else:
                with tc.For_i(0, repeats, 1):
                    _full_body()

    nc.compile()
    return nc


def _prepare(energy, cell_ids, position_weights, W1, b1, ln1_g, ln1_b,
             W2, b2, ln2_g, ln2_b, W3, b3):
    """Host-side prep: shard + fold scatter/gather/LN into the weights.
    Returns (bias_flags_key, per-core input maps)."""
    energy = np.asarray(energy, dtype=np.float32)
    cell_ids = np.asarray(cell_ids)
    position_weights = np.asarray(position_weights, dtype=np.float32)
    W1 = np.asarray(W1, dtype=np.float32)
    W2 = np.asarray(W2, dtype=np.float32)
    W3 = np.asarray(W3, dtype=np.float32)
    b1 = np.asarray(b1, dtype=np.float32)
    b2 = np.asarray(b2, dtype=np.float32)
    b3 = np.asarray(b3, dtype=np.float32)
    ln1_g = np.asarray(ln1_g, dtype=np.float32)
    ln1_b = np.asarray(ln1_b, dtype=np.float32)
    ln2_g = np.asarray(ln2_g, dtype=np.float32)
    ln2_b = np.asarray(ln2_b, dtype=np.float32)

    ids = cell_ids.astype(np.int64)
    # scatter surface[:, ids] = (energy * w).T  ==  row-gather of W1 at ids
    # (ids is a permutation: fill=arange per the problem spec)
    w = position_weights.reshape(-1)[ids]
    W1f = w[:, None] * W1[ids]

    # fold LN affine params into the next layer (exact fp32 host math):
    # (xn*g + lb) @ W + b  ==  xn @ (diag(g) W) + (lb @ W + b)
    W2f = ln1_g[:, None] * W2
    b2f = ln1_b @ W2 + b2
    W3f = ln2_g[:, None] * W3
    b3f = ln2_b @ W3 + b3

    # fold the LN mean subtraction into the weights by column-centering:
    # (h - mu 1) @ W == h @ (W - ones colmean(W))
    W2c = W2f - W2f.mean(axis=0, keepdims=True) * 1.0
    W3c = W3f - W3f.mean(axis=0, keepdims=True) * 1.0

    with_b1 = bool(np.any(b1 != 0.0))
    with_b2 = bool(np.any(b2f != 0.0))
    with_b3 = bool(np.any(b3f != 0.0))
    key = (with_b1, with_b2, with_b3)

    base = {
        "w1": W1f.astype(np.float16),
        "w2": W2c.astype(np.float16),
        "w3": W3c.astype(np.float16),
    }
    if with_b1:
        base["b1"] = b1.astype(np.float16)
    if with_b2:
        base["b2"] = b2f.astype(np.float16)
    if with_b3:
        base["b3"] = b3f

    e16 = energy.astype(np.float16)
    in_maps = [
        {**base, "e": np.ascontiguousarray(e16[:, c * BC:(c + 1) * BC])}
        for c in range(N_CORES)
    ]
    return key, in_maps


def kernel(energy, cell_ids, position_weights, W1, b1, ln1_g, ln1_b,
           W2, b2, ln2_g, ln2_b, W3, b3):
    key, in_maps = _prepare(energy, cell_ids, position_weights, W1, b1,
                            ln1_g, ln1_b, W2, b2, ln2_g, ln2_b, W3, b3)
    if key not in _PROGRAM_CACHE:
        _PROGRAM_CACHE[key] = _build_program(*key)
    nc = _PROGRAM_CACHE[key]
    res = run_bass_kernel_spmd(nc, in_maps, core_ids=list(range(N_CORES)))
    global _LAST_EXEC_NS
    if res.exec_time_ns is not None:
        _LAST_EXEC_NS = res.exec_time_ns
    return np.concatenate([r["out"] for r in res.results], axis=0)


# revision 10
# speedup vs baseline: 1.5159x; 1.0296x over previous
"""Bass/Trainium2 kernel for nn_EnergyOutputCollector.

Math (per batch row b):
    w[c]      = position_weights.flat[cell_ids[c]]
    surface   = scatter(energy * w) -> [B, 1024]   (cell_ids is a permutation)
    h1 = LN(gelu_tanh(surface @ W1 + b1)) * g1 + bb1
    h2 = LN(gelu_tanh(h1 @ W2 + b2)) * g2 + bb2
    out = h2 @ W3 + b3

Strategy:
  - Data-parallel: batch (16384) split across 8 NeuronCores (2048 each).
  - The scatter + position-weight gather + LN affine params fold into the
    weights on the host in exact fp32:
        W1' = (w[:,None] * W1[cell_ids])          (scatter == row gather of W1)
        W2' = diag(g1) @ W2,  bias2' = bb1 @ W2 + b2
        W3' = diag(g2) @ W3,  bias3' = bb2 @ W3 + b3
  - Device: pure 3-layer MLP in fp16 (PE full rate, ~1e-3 rel err overall),
    fp32 PSUM accumulation, gelu on ScalarE straight from PSUM (one op per
    2048-wide layer output, spanning 4 PSUM banks), LayerNorm stats via
    bn_stats/bn_aggr on VectorE, one batched fp16 SBUF->SBUF DMA transpose
    per layer output (contraction dim must sit on partitions for the next
    matmul).
  - Batch tiles processed layer-major in groups of 4 so ScalarE activation
    ops of the same function (gelu vs sqrt table sets) batch together --
    ACT table-set swaps cost ~1.3-2.7us each.
"""

import numpy as np

import concourse.bass as bass
import concourse.mybir as mybir
import concourse.tile as tile
from concourse import bacc
from concourse.bass_utils import run_bass_kernel_spmd

N_CORES = 8
SURF = 1024
HID = 2048
INTER = 2048
OUT = 768
BATCH = 16384
BC = BATCH // N_CORES          # batch per core
MT = BC // 128                 # m-tiles per core (16)
GROUP = 4                      # m-tiles per layer-major group
EPS = 1e-5

F = mybir.ActivationFunctionType
ALU = mybir.AluOpType
F16 = mybir.dt.float16
F32 = mybir.dt.float32

_PROGRAM_CACHE: dict = {}
_LAST_EXEC_NS = None


def _build_program(with_b1: bool, with_b2: bool, with_b3: bool, repeats: int = 1,
                   ps_w: int = 2048, ps_bufs: int = 2, group: int = GROUP):
    assert MT % group == 0, (MT, group)
    nc = bacc.Bacc(None, target_bir_lowering=False, debug=False)

    e = nc.dram_tensor("e", [SURF, BC], F16, kind="ExternalInput")
    w1 = nc.dram_tensor("w1", [SURF, HID], F16, kind="ExternalInput")
    w2 = nc.dram_tensor("w2", [HID, INTER], F16, kind="ExternalInput")
    w3 = nc.dram_tensor("w3", [INTER, OUT], F16, kind="ExternalInput")
    b1d = nc.dram_tensor("b1", [HID], F16, kind="ExternalInput") if with_b1 else None
    b2d = nc.dram_tensor("b2", [INTER], F16, kind="ExternalInput") if with_b2 else None
    b3d = nc.dram_tensor("b3", [OUT], F32, kind="ExternalInput") if with_b3 else None
    out = nc.dram_tensor("out", [BC, OUT], F32, kind="ExternalOutput")

    # K-on-partitions layouts for matmul operands
    et = e.rearrange("(ko p) b -> p ko b", p=128)      # [128, 8, BC]
    w1t = w1.rearrange("(ko p) n -> p ko n", p=128)    # [128, 8, HID]
    w2t = w2.rearrange("(ko p) n -> p ko n", p=128)    # [128, 16, INTER]
    w3t = w3.rearrange("(ko p) n -> p ko n", p=128)    # [128, 16, OUT]

    with tile.TileContext(nc) as tc:
        with (
            tc.tile_pool(name="weights", bufs=1) as wp,
            tc.tile_pool(name="consts", bufs=1) as cp,
            tc.tile_pool(name="etiles", bufs=group + 2) as ep,
            tc.tile_pool(name="acts", bufs=group + 2) as hp,
            tc.tile_pool(name="actsT", bufs=group + 2) as tp,
            tc.tile_pool(name="stats", bufs=2 * group) as sp,
            tc.tile_pool(name="outs", bufs=min(group, 4)) as op,
            tc.tile_pool(name="psum", bufs=ps_bufs, space="PSUM") as pp,
        ):
            w1_sb = wp.tile([128, 8, HID], F16, tag="w1")
            w2_sb = wp.tile([128, 16, INTER], F16, tag="w2")
            w3_sb = wp.tile([128, 16, OUT], F16, tag="w3")
            nc.sync.dma_start(w1_sb[:], w1t)
            nc.sync.dma_start(w2_sb[:], w2t)
            nc.sync.dma_start(w3_sb[:], w3t)

            eps_t = cp.tile([128, 1], F32, tag="eps")
            nc.vector.memset(eps_t[:], EPS)

            def bias_bc(dram_vec, n, tag, dt=F32):
                t = cp.tile([128, n], dt, tag=tag)
                src = bass.AP(
                    tensor=dram_vec.tensor,
                    offset=dram_vec.offset,
                    ap=[[0, 128]] + list(dram_vec.ap),
                )
                nc.sync.dma_start(t[:], src)
                return t

            b1_sb = bias_bc(b1d[:], HID, "b1", F16) if with_b1 else None
            b2_sb = bias_bc(b2d[:], INTER, "b2", F16) if with_b2 else None
            b3_sb = bias_bc(b3d[:], OUT, "b3") if with_b3 else None

            def mlp_layer(lhsT_of, w_sb, nk, bias_sb, tiles):
                """matmul over nk k-chunks + optional bias + gelu, for each
                tile in `tiles`; returns gelu'd fp16 [128, 2048] tiles."""
                hs = {}
                for t in tiles:
                    h = hp.tile([128, 2048], F16, tag="h")
                    for p0 in range(0, 2048, ps_w):
                        ps = pp.tile([128, ps_w], F32, tag="ps")
                        for n in range(ps_w // 512):
                            psl = slice(n * 512, (n + 1) * 512)
                            nsl = slice(p0 + n * 512, p0 + (n + 1) * 512)
                            for k in range(nk):
                                nc.tensor.matmul(
                                    ps[:, psl], lhsT_of(t, k), w_sb[:, k, nsl],
                                    start=(k == 0), stop=(k == nk - 1),
                                )
                        hsl = slice(p0, p0 + ps_w)
                        if bias_sb is not None:
                            nc.vector.tensor_add(
                                out=ps[:], in0=ps[:], in1=bias_sb[:, hsl]
                            )
                        nc.scalar.activation(
                            out=h[:, hsl], in_=ps[:], func=F.Gelu_apprx_tanh
                        )
                    hs[t] = h
                return hs

            def layernorm_transpose(hs, tiles):
                """LN (in place) then batched transpose; returns hT tiles."""
                mvs = {}
                for t in tiles:
                    stats = sp.tile([128, 4, 6], F32, tag="stats")
                    for n in range(4):
                        nc.vector.bn_stats(
                            out=stats[:, n, :], in_=hs[t][:, n * 512:(n + 1) * 512]
                        )
                    mv = sp.tile([128, 2], F32, tag="mv")
                    nc.vector.bn_aggr(out=mv[:], in_=stats[:])
                    mvs[t] = mv
                rs = {}
                for t in tiles:   # batched: one ACT table-set swap per group
                    r = sp.tile([128, 1], F32, tag="rstd")
                    nc.scalar.activation(
                        out=r[:], in_=mvs[t][:, 1:2], func=F.Sqrt, bias=eps_t[:]
                    )
                    rs[t] = r
                hts = {}
                for t in tiles:
                    nc.vector.reciprocal(out=rs[t][:], in_=rs[t][:])
                    nc.vector.tensor_scalar(
                        out=hs[t][:], in0=hs[t][:],
                        scalar1=mvs[t][:, 0:1], scalar2=rs[t][:],
                        op0=ALU.subtract, op1=ALU.mult,
                    )
                    ht = tp.tile([128, 16, 128], F16, tag="ht")
                    nc.sync.dma_start_transpose(ht[:], hs[t][:])
                    hts[t] = ht
                return hts

            def _full_body():
                for g in range(MT // group):
                    tiles = list(range(g * group, (g + 1) * group))

                    e_sbs = {}
                    for t in tiles:
                        e_sb = ep.tile([128, 8, 128], F16, tag="e")
                        nc.sync.dma_start(
                            e_sb[:], et[:, :, t * 128:(t + 1) * 128]
                        )
                        e_sbs[t] = e_sb

                    h1 = mlp_layer(
                        lambda t, k: e_sbs[t][:, k, :], w1_sb, 8, b1_sb, tiles
                    )
                    h1T = layernorm_transpose(h1, tiles)
                    h2 = mlp_layer(
                        lambda t, k: h1T[t][:, k, :], w2_sb, 16, b2_sb, tiles
                    )
                    h2T = layernorm_transpose(h2, tiles)

                    for t in tiles:
                        if ps_w >= 1024:
                            ps = pp.tile([128, ps_w], F32, tag="ps", name="ps_l3")
                        else:
                            ps = pp.tile([128, 1024], F32, tag="ps3", name="ps_l3")
                        for n in range(2):
                            bsl = slice(n * 512, n * 512 + 384)
                            nsl = slice(n * 384, (n + 1) * 384)
                            for k in range(16):
                                nc.tensor.matmul(
                                    ps[:, bsl], h2T[t][:, k, :], w3_sb[:, k, nsl],
                                    start=(k == 0), stop=(k == 15),
                                )
                        ps3 = ps[:, :1024].rearrange("p (b f) -> p b f", f=512)[:, :2, :384]
                        o_sb = op.tile([128, 2, 384], F32, tag="o")
                        if b3_sb is not None:
                            nc.vector.tensor_tensor(
                                o_sb[:], ps3,
                                b3_sb[:].rearrange("p (b f) -> p b f", f=384),
                                ALU.add,
                            )
                        else:
                            nc.vector.tensor_copy(o_sb[:], ps3)
                        nc.sync.dma_start(
                            out[t * 128:(t + 1) * 128, :],
                            o_sb[:].rearrange("p b f -> p (b f)"),
                        )

            if repeats == 1:
                _full_body()
            else:
                # For_i drains all engines at each loop edge; unrolling U
                # bodies per iteration amortizes that barrier and lets
                # consecutive bodies pipeline through the tile pools.
                U = 2 if repeats % 2 == 0 else 1
                with tc.For_i(0, repeats // U, 1):
                    for _ in range(U):
                        _full_body()

    nc.compile()
    return nc


def _prepare(energy, cell_ids, position_weights, W1, b1, ln1_g, ln1_b,
             W2, b2, ln2_g, ln2_b, W3, b3):
    """Host-side prep: shard + fold scatter/gather/LN-affine into weights.
    Returns (bias_flags_key, per-core input maps)."""
    energy = np.asarray(energy, dtype=np.float32)
    cell_ids = np.asarray(cell_ids)
    position_weights = np.asarray(position_weights, dtype=np.float32)
    W1 = np.asarray(W1, dtype=np.float32)
    W2 = np.asarray(W2, dtype=np.float32)
    W3 = np.asarray(W3, dtype=np.float32)
    b1 = np.asarray(b1, dtype=np.float32)
    b2 = np.asarray(b2, dtype=np.float32)
    b3 = np.asarray(b3, dtype=np.float32)
    ln1_g = np.asarray(ln1_g, dtype=np.float32)
    ln1_b = np.asarray(ln1_b, dtype=np.float32)
    ln2_g = np.asarray(ln2_g, dtype=np.float32)
    ln2_b = np.asarray(ln2_b, dtype=np.float32)

    ids = cell_ids.astype(np.int64)
    # scatter surface[:, ids] = (energy * w).T  ==  row-gather of W1 at ids
    # (ids is a permutation: fill=arange per the problem spec)
    w = position_weights.reshape(-1)[ids]
    W1f = w[:, None] * W1[ids]

    # fold LN affine params into the next layer (exact fp32 host math):
    # (xn*g + lb) @ W + b  ==  xn @ (diag(g) W) + (lb @ W + b)
    W2f = ln1_g[:, None] * W2
    b2f = ln1_b @ W2 + b2
    W3f = ln2_g[:, None] * W3
    b3f = ln2_b @ W3 + b3

    with_b1 = bool(np.any(b1 != 0.0))
    with_b2 = bool(np.any(b2f != 0.0))
    with_b3 = bool(np.any(b3f != 0.0))
    key = (with_b1, with_b2, with_b3)

    base = {
        "w1": W1f.astype(np.float16),
        "w2": W2f.astype(np.float16),
        "w3": W3f.astype(np.float16),
    }
    if with_b1:
        base["b1"] = b1.astype(np.float16)
    if with_b2:
        base["b2"] = b2f.astype(np.float16)
    if with_b3:
        base["b3"] = b3f

    e16 = energy.astype(np.float16)
    in_maps = [
        {**base, "e": np.ascontiguousarray(e16[:, c * BC:(c + 1) * BC])}
        for c in range(N_CORES)
    ]
    return key, in_maps


def kernel(energy, cell_ids, position_weights, W1, b1, ln1_g, ln1_b,
           W2, b2, ln2_g, ln2_b, W3, b3):
    key, in_maps = _prepare(energy, cell_ids, position_weights, W1, b1,
                            ln1_g, ln1_b, W2, b2, ln2_g, ln2_b, W3, b3)
    if key not in _PROGRAM_CACHE:
        _PROGRAM_CACHE[key] = _build_program(*key)
    nc = _PROGRAM_CACHE[key]
    res = run_bass_kernel_spmd(nc, in_maps, core_ids=list(range(N_CORES)))
    global _LAST_EXEC_NS
    if res.exec_time_ns is not None:
        _LAST_EXEC_NS = res.exec_time_ns
    return np.concatenate([r["out"] for r in res.results], axis=0)

